# revision 1
# baseline (speedup 1.0000x reference)
"""Single-head causal attention (B=4, T=2048, C=1024, fp32) on 8 Trainium2 cores.

Sharding: core = (batch b = core//2, half h = core%2). Each core computes the
attention output for 1024 query rows of one batch (4 q-blocks of 256 rows,
chosen so every core has an identical, perfectly-balanced causal schedule).

Per-core schedule: 4 slots with [16, 12, 8, 4] column-tile units (128 cols
each) = 40 units everywhere. Slot -> q-block assignment (host-side data):
  h=0 -> g = [7, 4, 3, 0]   (needs [16, 10, 8, 2] col tiles; dummies masked)
  h=1 -> g = [6, 5, 2, 1]   (needs [14, 12, 6, 4])
Causality + dummy-unit suppression are handled by multiplicative {0,1} masks
applied after exp. Logits are O(1) for this problem (|S|/sqrt(C) < ~4), so
softmax needs no running-max: out = sum_j exp(S_j) V_j / sum_j exp(S_j).

All matmuls run as float32r (full PE rate at N>=256, ~tf32 precision).
"""

import os
import sys

import numpy as np

for _p in ("/opt/trn_rl_repo", os.path.expanduser("~/.axon_site/_ro/trn_rl_repo")):
    if os.path.isdir(_p) and _p not in sys.path:
        sys.path.insert(0, _p)

B, T, C = 4, 2048, 1024
QBLK = 256            # q rows per slot
NSLOT = 4
SLOT_UNITS = [16, 12, 8, 4]
NUNITS = sum(SLOT_UNITS)          # 40
ASSIGN = {0: [7, 4, 3, 0], 1: [6, 5, 2, 1]}
SCALE = float(C) ** -0.5

_CACHE = {}


def _build_nc():
    import concourse.tile as tile
    from concourse import bacc, mybir
    from contextlib import ExitStack

    f32 = mybir.dt.float32
    f32r = mybir.dt.float32r
    bf16 = mybir.dt.bfloat16
    Exp = mybir.ActivationFunctionType.Exp
    Copy = mybir.ActivationFunctionType.Copy

    nc = bacc.Bacc("TRN2", target_bir_lowering=False, debug=False)

    xT = nc.dram_tensor("xT", [C, T], f32r, kind="ExternalInput").ap()
    xqT = nc.dram_tensor("xqT", [C, 1024], f32r, kind="ExternalInput").ap()
    wkT = nc.dram_tensor("wkT", [C, C], f32r, kind="ExternalInput").ap()
    wqT = nc.dram_tensor("wqT", [C, C], f32r, kind="ExternalInput").ap()
    wvT = nc.dram_tensor("wvT", [C, C], f32r, kind="ExternalInput").ap()
    masks = nc.dram_tensor("masks", [NUNITS, 128, QBLK], bf16, kind="ExternalInput").ap()
    out = nc.dram_tensor("out", [1024, C], f32, kind="ExternalOutput").ap()

    r = lambda ap: ap

    def load_chunked(pool, name, dram_ap, cols, bufs=None):
        """DMA a [1024, cols] DRAM slice into one [128, 8*cols] tile
        (cin-chunk ci lives at free offset ci*cols)."""
        t = pool.tile([128, 8 * cols], f32r, tag=name.rstrip("0123456789_") or name,
                      name=name, bufs=bufs)
        nc.sync.dma_start(
            out=t[:].rearrange("p (a m) -> p a m", a=8),
            in_=dram_ap.rearrange("(a p) m -> p a m", p=128),
        )
        return t

    with tile.TileContext(nc) as tc, ExitStack() as ctx:
        # ---- persistent SBUF arrays (160 KB/partition) ------------------
        kt_pool = ctx.enter_context(tc.tile_pool(name="ktp", bufs=1))
        v_pool = ctx.enter_context(tc.tile_pool(name="vp", bufs=1))
        qt_pool = ctx.enter_context(tc.tile_pool(name="qtp", bufs=1))
        misc_pool = ctx.enter_context(tc.tile_pool(name="miscp", bufs=1))

        # KT[ci]: [128 co, 2048 k] for co-chunk ci; contraction operand of S^T
        KT = [kt_pool.tile([128, T], f32r, tag=f"kt{i}", name=f"kt{i}") for i in range(8)]
        # V[kc]: [128 k, 1024 co] for k-chunk kc
        V = [v_pool.tile([128, C], f32r, tag=f"v{i}", name=f"v{i}") for i in range(16)]
        # QT[ci]: [128 co, 1024 q] (q in slot order)
        QT = [qt_pool.tile([128, 1024], f32r, tag=f"qt{i}", name=f"qt{i}") for i in range(8)]
        ones_f = misc_pool.tile([128, 2], f32, name="ones_f")
        nc.vector.memset(ones_f[:], 1.0)
        ones = misc_pool.tile([128, 2], f32r, name="ones")
        nc.scalar.copy(ones[:], ones_f[:])

        # ---- phase Q: QT[co][:, s] = sum_ci WqT[ci, co].T @ xq[ci, s] ---
        with tc.tile_pool(name="wqp", bufs=1) as wq_pool, \
             tc.tile_pool(name="xqp", bufs=1) as xq_pool, \
             tc.psum_pool(name="pq", bufs=4) as pq:
            wq = load_chunked(wq_pool, "wq", wqT, C)          # 32 KB/part
            for s in range(NSLOT):
                xq = load_chunked(xq_pool, "xq", xqT[:, QBLK * s:QBLK * (s + 1)], QBLK)
                for co in range(8):
                    ps = pq.tile([128, QBLK], f32, tag="proj", name=f"qps{s}_{co}")
                    for ci in range(8):
                        nc.tensor.matmul(
                            ps[:],
                            r(wq[:, C * ci + 128 * co: C * ci + 128 * (co + 1)]),
                            r(xq[:, QBLK * ci:QBLK * (ci + 1)]),
                            start=(ci == 0), stop=(ci == 7),
                        )
                    nc.scalar.copy(QT[co][:, QBLK * s:QBLK * (s + 1)], ps[:])

        # ---- phase V: V = x @ Wv^T  (layout [k, co]) --------------------
        with tc.tile_pool(name="wvp", bufs=1) as wv_pool, \
             tc.tile_pool(name="xwv", bufs=2) as xw_pool, \
             tc.psum_pool(name="pv", bufs=4) as pv:
            wv = load_chunked(wv_pool, "wv", wvT, C)          # 32 KB/part
            for kc in range(16):              # 16 windows x 128 k
                xw = load_chunked(xw_pool, "xwv", xT[:, 128 * kc:128 * (kc + 1)], 128,
                                  bufs=3)
                for half in range(2):
                    ps = pv.tile([128, 512], f32, tag="proj", name=f"vps{kc}_{half}")
                    for ci in range(8):
                        nc.tensor.matmul(
                            ps[:],
                            r(xw[:, 128 * ci:128 * (ci + 1)]),
                            r(wv[:, C * ci + 512 * half: C * ci + 512 * (half + 1)]),
                            start=(ci == 0), stop=(ci == 7),
                        )
                    nc.scalar.copy(V[kc][:, 512 * half:512 * (half + 1)], ps[:])

        # ---- phase K: KT = Wk @ x^T  (layout [co, k]), Wk in co-halves --
        with tc.tile_pool(name="wkp", bufs=1) as wk_pool, \
             tc.tile_pool(name="xwk", bufs=2) as xk_pool, \
             tc.psum_pool(name="pk", bufs=4) as pk:
            for half in range(2):
                wkh = load_chunked(wk_pool, f"wk{half}",
                                   wkT[:, 512 * half:512 * (half + 1)], 512)
                for kw in range(8):           # 8 windows x 256 k (re-streamed)
                    xw = load_chunked(xk_pool, f"xwk{half}_{kw}",
                                      xT[:, 256 * kw:256 * (kw + 1)], 256, bufs=3)
                    for co4 in range(4):
                        co = 4 * half + co4
                        ps = pk.tile([128, 256], f32, tag="proj", name=f"kps{half}_{kw}_{co4}")
                        for ci in range(8):
                            nc.tensor.matmul(
                                ps[:],
                                r(wkh[:, 512 * ci + 128 * co4: 512 * ci + 128 * (co4 + 1)]),
                                r(xw[:, 256 * ci:256 * (ci + 1)]),
                                start=(ci == 0), stop=(ci == 7),
                            )
                        nc.scalar.copy(KT[co][:, 256 * kw:256 * (kw + 1)], ps[:])

        # ---- attention --------------------------------------------------
        with tc.tile_pool(name="maskp", bufs=2) as mask_pool, \
             tc.tile_pool(name="ptp", bufs=2) as pt_pool, \
             tc.tile_pool(name="outp", bufs=2) as out_pool, \
             tc.tile_pool(name="linvp", bufs=2) as linv_pool, \
             tc.psum_pool(name="sp", bufs=2) as sp, \
             tc.psum_pool(name="op", bufs=1) as op, \
             tc.psum_pool(name="lp", bufs=1) as lp:
            u0 = 0
            for s in range(NSLOT):
                n = SLOT_UNITS[s]
                mslot = mask_pool.tile([128, n * QBLK], bf16, tag="m", name=f"mslot{s}")
                nc.sync.dma_start(
                    out=mslot[:].rearrange("p (u m) -> p u m", u=n),
                    in_=masks[u0:u0 + n, :, :].rearrange("u p m -> p u m"),
                )
                o_ps = [op.tile([128, C], f32, tag=f"o{qc}", name=f"o{qc}_{s}") for qc in range(2)]
                l_ps = [lp.tile([128, 2], f32, tag=f"l{qc}", name=f"l{qc}_{s}") for qc in range(2)]
                for j in range(n):
                    s_ps = sp.tile([128, QBLK], f32, tag="s", name=f"s{s}_{j}")
                    for ci in range(8):
                        nc.tensor.matmul(
                            s_ps[:],
                            r(KT[ci][:, 128 * j:128 * (j + 1)]),
                            r(QT[ci][:, QBLK * s:QBLK * (s + 1)]),
                            start=(ci == 0), stop=(ci == 7),
                        )
                    sm_t = pt_pool.tile([128, QBLK], f32, tag="sm", name=f"sm{s}_{j}")
                    nc.vector.tensor_add(sm_t[:], s_ps[:], mslot[:, QBLK * j:QBLK * (j + 1)])
                    pm_t = pt_pool.tile([128, QBLK], f32r, tag="pm", name=f"pm{s}_{j}")
                    nc.scalar.activation(pm_t[:], sm_t[:], Exp, scale=SCALE)
                    first, last = (j == 0), (j == n - 1)
                    for qc in range(2):
                        lhsT = r(pm_t[:, 128 * qc:128 * (qc + 1)])
                        nc.tensor.matmul(o_ps[qc][:, 0:512], lhsT, r(V[j][:, 0:512]),
                                         start=first, stop=last)
                        nc.tensor.matmul(o_ps[qc][:, 512:1024], lhsT, r(V[j][:, 512:1024]),
                                         start=first, stop=last)
                        nc.tensor.matmul(l_ps[qc][:], lhsT, r(ones[:]),
                                         start=first, stop=last)
                u0 += n
                for qc in range(2):
                    linv = linv_pool.tile([128, 1], f32, tag="linv", name=f"linv{s}_{qc}")
                    nc.vector.reciprocal(linv[:], l_ps[qc][:, 0:1])
                    o_sb = out_pool.tile([128, C], f32, tag="ost", name=f"ost{s}_{qc}")
                    nc.scalar.activation(o_sb[:], o_ps[qc][:], Copy, scale=linv[:])
                    nc.sync.dma_start(
                        out=out[QBLK * s + 128 * qc: QBLK * s + 128 * (qc + 1), :],
                        in_=o_sb[:],
                    )
    nc.finalize()
    return nc


def _masks_for_half(h):
    import ml_dtypes
    m = np.zeros((NUNITS, 128, QBLK), ml_dtypes.bfloat16)
    u = 0
    for s in range(NSLOT):
        g = ASSIGN[h][s]
        for j in range(SLOT_UNITS[s]):
            ks = 128 * j + np.arange(128)[:, None]
            qs = 256 * g + np.arange(QBLK)[None, :]
            m[u] = np.where(ks <= qs, 0.0, -30000.0).astype(ml_dtypes.bfloat16)
            u += 1
    return m


def _get_built():
    if "nc" not in _CACHE:
        _CACHE["nc"] = _build_nc()
        _CACHE["masks"] = {h: _masks_for_half(h) for h in (0, 1)}
    return _CACHE["nc"], _CACHE["masks"]


def kernel(x, Wk, Wq, Wv, **_ignored):
    from concourse.bass_utils import run_bass_kernel_spmd

    nc, mks = _get_built()
    x = np.ascontiguousarray(np.asarray(x, np.float32))
    wkT = np.ascontiguousarray(np.asarray(Wk, np.float32).T)
    wqT = np.ascontiguousarray(np.asarray(Wq, np.float32).T)
    wvT = np.ascontiguousarray(np.asarray(Wv, np.float32).T)

    in_maps = []
    for core in range(8):
        b, h = core // 2, core % 2
        xT_b = np.ascontiguousarray(x[b].T)
        gs = ASSIGN[h]
        xqT = np.ascontiguousarray(
            np.concatenate([xT_b[:, 256 * g:256 * (g + 1)] for g in gs], axis=1)
        )
        in_maps.append({
            "xT": xT_b, "xqT": xqT,
            "wkT": wkT, "wqT": wqT, "wvT": wvT,
            "masks": mks[h],
        })

    res = run_bass_kernel_spmd(nc, in_maps, core_ids=list(range(8)))
    _CACHE["last_res"] = res

    out = np.empty((B, T, C), np.float32)
    for core in range(8):
        b, h = core // 2, core % 2
        o = res.results[core]["out"]
        for s, g in enumerate(ASSIGN[h]):
            out[b, 256 * g:256 * (g + 1), :] = o[256 * s:256 * (s + 1), :]
    return out

